# revision 1
# baseline (speedup 1.0000x reference)
"""VQ codebook-lookup kernel for TRN2, data-parallel over batch on 8 NeuronCores.

Reference computation (per batch b with class c[b]):
  z = z_e_x[b] viewed as [N=4096, D=256] (D innermost)
  cb = embedding[c[b]*512:(c[b]+1)*512]            # [K=512, D]
  idx[n] = argmin_k ||z[n] - cb[k]||^2 = argmax_k (z[n].cb[k] - ||cb[k]||^2/2)
  out[n] = cb[idx[n]]

Device strategy per core (4 batches):
  - scores S[n,k] via TensorE: 3-pass split-bf16 matmul (exact-argmax precision)
    with z kept in its natural [D, N] layout as the stationary operand
  - bias (-||cb||^2/2) added and row-max computed in one fused DVE op
  - one-hot = (S == max) on DVE, transposed on TensorE, then a bf16 matmul
    gathers the selected codewords
Host side only reindexes/splits operands and reassembles the output.
"""

import sys

sys.path.insert(0, "/opt/trn_rl_repo")

import numpy as np

B, D, HH, WW = 32, 256, 64, 64
N = HH * WW            # 4096 positions per batch
K = 512                # codes per class
NUM_CLASSES = 60
NCORES = 8
BPC = B // NCORES      # batches per core
NT = N // 128          # 32 n-tiles per batch

_CACHE = {}

# set by test harness to request an NTFF profile
TRACE = False
LAST_EXEC_NS = None


def _build(bpc=BPC, nt=NT, repeat=1, dist_passes=3, tail="full", gather="bf16",
           psum_cfg=(3, 1, 1), sco_bufs=3, z_chunks=1, zb_bufs=2):
    # tail: "full" | "nored" (skip add/max/eq/transp/gather; out from psS)
    #       | "noeq" (keep add+max, skip eq/transp/gather)
    from concourse import bacc, tile, mybir
    import ml_dtypes

    f32 = mybir.dt.float32
    bf16 = mybir.dt.bfloat16
    Alu = mybir.AluOpType

    nc = bacc.Bacc("TRN2", target_bir_lowering=False)

    z1_ext = nc.declare_dram_parameter("z1", [bpc, 128, 2, N], bf16, isOutput=False)
    z2_ext = nc.declare_dram_parameter("z2", [bpc, 128, 2, N], bf16, isOutput=False)
    fp8 = mybir.dt.float8e4
    ct1_ext = nc.declare_dram_parameter("ct1", [bpc, 128, 2, K], bf16, isOutput=False)
    ct2_ext = nc.declare_dram_parameter("ct2", [bpc, 128, 2, K], bf16, isOutput=False)
    if gather == "dr":
        # gather codebook as two scaled fp8 halves (hi, lo): cb*64 = hi + lo
        cbg_ext = nc.declare_dram_parameter("cbg", [bpc, 128, 2, 4, D], fp8,
                                            isOutput=False)
    else:
        cbgb_ext = nc.declare_dram_parameter("cbgb", [bpc, 128, 4, D], bf16,
                                             isOutput=False)
    # bias duplicated along axis 2 so one paired [128, 2*K] DVE add covers 2 tiles
    bias_ext = nc.declare_dram_parameter("bias", [bpc, 128, 2, K], f32, isOutput=False)
    out_ext = nc.declare_dram_parameter("out", [bpc, 128, nt, D], bf16, isOutput=True)

    ident_dram = nc.inline_tensor(np.eye(128, dtype=ml_dtypes.bfloat16), name="ident")

    with tile.TileContext(nc) as tc:
        with (
            tc.tile_pool(name="const", bufs=1) as constp,
            tc.tile_pool(name="zb", bufs=zb_bufs) as zb,
            tc.tile_pool(name="cbp", bufs=2) as cbp,
            tc.tile_pool(name="outp", bufs=2) as outp,
            tc.tile_pool(name="sco", bufs=sco_bufs) as sco,
            tc.tile_pool(name="psS", bufs=psum_cfg[0], space="PSUM") as psSp,
            tc.tile_pool(name="psT", bufs=psum_cfg[1], space="PSUM") as psTp,
            tc.tile_pool(name="psQ", bufs=psum_cfg[2], space="PSUM") as psQp,
        ):
            ident = constp.tile([128, 128], bf16, tag="ident")
            nc.sync.dma_start(ident[:], ident_dram[:])

            for b in [bb for _ in range(repeat) for bb in range(bpc)]:
                z1 = zb.tile([128, 2, N], bf16, tag="z1")
                z2 = zb.tile([128, 2, N], bf16, tag="z2")
                ct1 = cbp.tile([128, 2, K], bf16, tag="ct1")
                ct2 = cbp.tile([128, 2, K], bf16, tag="ct2")
                bias = cbp.tile([128, 2, K], f32, tag="bias")
                # small codebook tensors first: the first matmul needs them
                nc.sync.dma_start(ct1[:], ct1_ext[b])
                nc.sync.dma_start(ct2[:], ct2_ext[b])
                if gather == "dr":
                    cbg = cbp.tile([128, 2, 4, D], fp8, tag="cbg")
                    nc.sync.dma_start(cbg[:], cbg_ext[b])
                else:
                    cbgb = cbp.tile([128, 4, D], bf16, tag="cbgb")
                    nc.sync.dma_start(cbgb[:], cbgb_ext[b])
                nc.sync.dma_start(bias[:], bias_ext[b])
                zc = N // z_chunks
                for ci in range(z_chunks):
                    nc.sync.dma_start(z1[:, :, ci * zc:(ci + 1) * zc],
                                      z1_ext[b, :, :, ci * zc:(ci + 1) * zc])
                    nc.sync.dma_start(z2[:, :, ci * zc:(ci + 1) * zc],
                                      z2_ext[b, :, :, ci * zc:(ci + 1) * zc])

                out_sb = outp.tile([128, nt, D], bf16, tag="out")

                # two n-tiles per iteration: halves the per-op overhead on the
                # elementwise engines (psS spans 2 PSUM banks, one slice each)
                for p in range(nt // 2):
                    psS = psSp.tile([128, 2, K], f32, tag="psS")
                    for h in range(2):
                        n0 = (2 * p + h) * 128
                        # dots = z1.c1 + z2.c1 + z1.c2  (split-fp32, exact argmax)
                        mms = [(z1, ct1, 0), (z1, ct1, 1), (z2, ct1, 0),
                               (z2, ct1, 1), (z1, ct2, 0), (z1, ct2, 1)]
                        mms = mms[:2 * dist_passes]
                        for i, (za, ca, cd) in enumerate(mms):
                            nc.tensor.matmul(psS[:, h, :], za[:, cd, n0:n0 + 128],
                                             ca[:, cd, :], start=(i == 0),
                                             stop=(i == len(mms) - 1))

                    if tail == "nored":
                        nc.scalar.copy(out_sb[:, 2 * p:2 * p + 2, :],
                                       psS[:, :, 0:D])
                        continue

                    # S = dots + bias; mx[n, h] = max_k S[n, h, k]
                    S_sb = sco.tile([128, 2, K], f32, tag="S")
                    mx = sco.tile([128, 2], f32, tag="mx")
                    nc.vector.tensor_add(S_sb[:], psS[:], bias[:])
                    nc.vector.reduce_max(mx[:], S_sb[:], axis=mybir.AxisListType.X)

                    if tail == "noeq":
                        nc.scalar.copy(out_sb[:, 2 * p:2 * p + 2, :],
                                       S_sb[:, :, 0:D])
                        continue

                    # one-hot of the argmax, bf16
                    oh = sco.tile([128, 2, K], bf16, tag="oh")
                    for h in range(2):
                        nc.vector.tensor_scalar(
                            out=oh[:, h, :], in0=S_sb[:, h, :],
                            scalar1=mx[:, h:h + 1], scalar2=None,
                            op0=Alu.is_equal,
                        )

                    if tail == "noTG":
                        nc.scalar.copy(out_sb[:, 2 * p:2 * p + 2, :],
                                       oh[:, :, 0:D])
                        continue

                    # transpose one-hot to [k, n] layout for the gather matmul
                    psT = psTp.tile([128, 2, K], bf16, tag="psT")
                    for h in range(2):
                        for j in range(4):
                            k0 = j * 128
                            nc.tensor.transpose(psT[:, h, k0:k0 + 128],
                                                oh[:, h, k0:k0 + 128], ident[:])
                    psQ = psQp.tile([128, 2, D], f32, tag="psQ")
                    if gather == "dr":
                        # cast one-hot to fp8 during the PSUM->SBUF copy
                        ohT = sco.tile([128, 2, K], fp8, tag="ohT")
                        nc.scalar.copy(ohT[:], psT[:])
                        # gather: out_q[n,d] = sum_k ohT[k,n] * (64*cb)[k,d]
                        # fp8 DoubleRow: contraction 256 per matmul
                        for h in range(2):
                            i = 0
                            for hl in range(2):
                                for jp in range(2):
                                    k0 = jp * 256
                                    nc.tensor.matmul(
                                        psQ[:, h, :],
                                        ohT[:, h, k0:k0 + 256].rearrange(
                                            "p (j m) -> p j m", j=2),
                                        cbg[:, hl, 2 * jp:2 * jp + 2, :],
                                        start=(i == 0), stop=(i == 3),
                                        perf_mode=mybir.MatmulPerfMode.DoubleRow)
                                    i += 1
                        # undo the 64x codebook scaling while copying out
                        nc.scalar.mul(out_sb[:, 2 * p:2 * p + 2, :], psQ[:],
                                      1.0 / 64.0)
                    else:
                        ohT = sco.tile([128, 2, K], bf16, tag="ohT")
                        nc.scalar.copy(ohT[:], psT[:])
                        for h in range(2):
                            for j in range(4):
                                k0 = j * 128
                                nc.tensor.matmul(psQ[:, h, :],
                                                 ohT[:, h, k0:k0 + 128],
                                                 cbgb[:, j, :], start=(j == 0),
                                                 stop=(j == 3))
                        nc.scalar.copy(out_sb[:, 2 * p:2 * p + 2, :], psQ[:])

                nc.sync.dma_start(out_ext[b], out_sb[:])

    nc.compile()
    return nc


def _get_nc():
    if "nc" not in _CACHE:
        _CACHE["nc"] = _build()
    return _CACHE["nc"]


def _prep_in_maps(z_e_x, c, embedding):
    import ml_dtypes

    bf = ml_dtypes.bfloat16

    z = np.ascontiguousarray(np.asarray(z_e_x), dtype=np.float32)      # [B, D, H, W]
    cls = np.asarray(c).astype(np.int64)                               # [B]
    emb = np.ascontiguousarray(np.asarray(embedding), dtype=np.float32)

    zf = z.reshape(B, D, N)                                            # [B, 256, 4096]
    z1 = zf.astype(bf)
    z2 = (zf - z1.astype(np.float32)).astype(bf)
    # [B, 256, N] -> [B, 128, 2, N] with d = cd*128 + p
    z1 = np.ascontiguousarray(z1.reshape(B, 2, 128, N).transpose(0, 2, 1, 3))
    z2 = np.ascontiguousarray(z2.reshape(B, 2, 128, N).transpose(0, 2, 1, 3))

    cb = emb.reshape(NUM_CLASSES, K, D)[cls]                           # [B, 512, 256]
    cbT = np.ascontiguousarray(cb.transpose(0, 2, 1))                  # [B, 256, 512]
    ct1 = cbT.astype(bf)
    ct2 = (cbT - ct1.astype(np.float32)).astype(bf)
    ct1 = np.ascontiguousarray(ct1.reshape(B, 2, 128, K).transpose(0, 2, 1, 3))
    ct2 = np.ascontiguousarray(ct2.reshape(B, 2, 128, K).transpose(0, 2, 1, 3))
    # gather operand: (64*cb) split into fp8 hi+lo, [B, 128, 2, 4, 256], k = j*128+p
    f8 = ml_dtypes.float8_e4m3
    cb64 = cb * np.float32(64.0)
    g_hi = cb64.astype(f8)
    g_lo = (cb64 - g_hi.astype(np.float32)).astype(f8)
    cbg = np.stack([g_hi, g_lo], axis=1).reshape(B, 2, 4, 128, D)
    cbg = np.ascontiguousarray(cbg.transpose(0, 3, 1, 2, 4))
    cbgb = np.ascontiguousarray(
        cb.astype(bf).reshape(B, 4, 128, D).transpose(0, 2, 1, 3))

    bias = -0.5 * np.sum(cb.astype(np.float64) ** 2, axis=2)           # [B, 512]
    bias_bc = np.ascontiguousarray(
        np.broadcast_to(bias.astype(np.float32)[:, None, None, :], (B, 128, 2, K))
    )

    in_maps = []
    for i in range(NCORES):
        s = slice(i * BPC, (i + 1) * BPC)
        in_maps.append({
            "z1": z1[s], "z2": z2[s], "ct1": ct1[s], "ct2": ct2[s],
            "cbg": cbg[s], "cbgb": cbgb[s], "bias": bias_bc[s],
        })
    return in_maps


def kernel(z_e_x, c, embedding):
    from concourse.bass_utils import run_bass_kernel_spmd

    global LAST_EXEC_NS

    in_maps = _prep_in_maps(z_e_x, c, embedding)
    nc = _get_nc()
    res = run_bass_kernel_spmd(nc, in_maps, core_ids=list(range(NCORES)),
                               trace=TRACE)
    LAST_EXEC_NS = res.exec_time_ns

    outs = np.concatenate([res.results[i]["out"].astype(np.float32)
                           for i in range(NCORES)], axis=0)
    # [B, 128, NT, D] -> [B, N, D] with n = t*128 + p
    out = outs.transpose(0, 2, 1, 3).reshape(B, N, D)
    return np.ascontiguousarray(out.reshape(B, HH, WW, D))



# revision 9
# speedup vs baseline: 1.7858x; 1.7858x over previous
"""VQ codebook-lookup kernel for TRN2, data-parallel over batch on 8 NeuronCores.

Reference computation (per batch b with class c[b]):
  z = z_e_x[b] viewed as [N=4096, D=256] (D innermost)
  cb = embedding[c[b]*512:(c[b]+1)*512]            # [K=512, D]
  idx[n] = argmin_k ||z[n] - cb[k]||^2 = argmax_k (z[n].cb[k] - ||cb[k]||^2/2)
  out[n] = cb[idx[n]]

Device strategy per core (4 batches):
  - scores S[n,k] on TensorE, all at a common 2^18 product scale (argmax is
    scale-invariant):
      * main term z1.c1 in float32r (the PE keeps 11 mantissa bits of f32r
        operands, measured; operands are pre-rounded to 11 bits on the host
        so the pass is exact) — 2 matmuls over the d=256 contraction
      * cross corrections z2.c1 + z1.c2 in one fp8-e4m3 DoubleRow pair per
        PE cell — 2 matmuls; one DR slot is stolen to add the bias low part
      * bias high part (r2n11 of -2^18*||cb||^2/2) via a contraction-1
        f32r matmul that starts the PSUM accumulation group
  - row-max on DVE straight from PSUM, one-hot = (S == max) on DVE
  - one-hot transposed on TensorE, then a bf16 matmul gathers the codewords
Host side only reindexes/splits/quantizes operands and reassembles the
output.  Simulated end-to-end argmin flips vs the f32 reference: 2 of 131072
(the exact-f32 baseline had 4).
"""

import sys

sys.path.insert(0, "/opt/trn_rl_repo")

import numpy as np

B, D, HH, WW = 32, 256, 64, 64
N = HH * WW            # 4096 positions per batch
K = 512                # codes per class
NUM_CLASSES = 60
NCORES = 8
BPC = B // NCORES      # batches per core
NT = N // 128          # 32 n-tiles per batch

_CACHE = {}

# set by test harness to request an NTFF profile
TRACE = False
LAST_EXEC_NS = None


def _build(bpc=BPC, nt=NT, repeat=1, psum_cfg=(3, 1, 1), sco_bufs=3, zb_bufs=2):
    from concourse import bacc, tile, mybir
    import ml_dtypes

    f32 = mybir.dt.float32
    f32r = mybir.dt.float32r
    bf16 = mybir.dt.bfloat16
    Alu = mybir.AluOpType

    nc = bacc.Bacc("TRN2", target_bir_lowering=False)

    fp8 = mybir.dt.float8e4
    z_ext = nc.declare_dram_parameter("z", [bpc, 128, 2, N], f32r, isOutput=False)
    ct_ext = nc.declare_dram_parameter("ct", [bpc, 128, 2, K], f32r, isOutput=False)
    zdr_ext = nc.declare_dram_parameter("zdr", [bpc, 128, 2, 2, N], fp8,
                                        isOutput=False)
    cdr_ext = nc.declare_dram_parameter("cdr", [bpc, 128, 2, 2, K], fp8,
                                        isOutput=False)
    cbgb_ext = nc.declare_dram_parameter("cbgb", [bpc, 128, 4, D], bf16,
                                         isOutput=False)
    biasrow_ext = nc.declare_dram_parameter("biasrow", [bpc, 1, K], f32r,
                                            isOutput=False)
    ones_ext = nc.declare_dram_parameter("ones", [1, 128], f32r, isOutput=False)
    out_ext = nc.declare_dram_parameter("out", [bpc, 128, nt, D], bf16, isOutput=True)

    ident_dram = nc.inline_tensor(np.eye(128, dtype=ml_dtypes.bfloat16), name="ident")

    with tile.TileContext(nc) as tc:
        with (
            tc.tile_pool(name="const", bufs=1) as constp,
            tc.tile_pool(name="zb", bufs=zb_bufs) as zb,
            tc.tile_pool(name="cbp", bufs=2) as cbp,
            tc.tile_pool(name="outp", bufs=2) as outp,
            tc.tile_pool(name="sco", bufs=sco_bufs) as sco,
            tc.tile_pool(name="psS", bufs=psum_cfg[0], space="PSUM") as psSp,
            tc.tile_pool(name="psT", bufs=psum_cfg[1], space="PSUM") as psTp,
            tc.tile_pool(name="psQ", bufs=psum_cfg[2], space="PSUM") as psQp,
        ):
            ident = constp.tile([128, 128], bf16, tag="ident")
            ones = constp.tile([1, 128], f32r, tag="ones")
            nc.sync.dma_start(ident[:], ident_dram[:])
            nc.sync.dma_start(ones[:], ones_ext[:])

            for b in [bb for _ in range(repeat) for bb in range(bpc)]:
                z = zb.tile([128, 2, N], f32r, tag="z")
                zdr = zb.tile([128, 2, 2, N], fp8, tag="zdr")
                ct = cbp.tile([128, 2, K], f32r, tag="ct")
                cdr = cbp.tile([128, 2, 2, K], fp8, tag="cdr")
                biasrow = cbp.tile([1, K], f32r, tag="biasrow")
                cbgb = cbp.tile([128, 4, D], bf16, tag="cbgb")
                # small codebook tensors first: the first matmul needs them
                nc.sync.dma_start(ct[:], ct_ext[b])
                nc.sync.dma_start(cdr[:], cdr_ext[b])
                nc.sync.dma_start(biasrow[:], biasrow_ext[b])
                nc.sync.dma_start(cbgb[:], cbgb_ext[b])
                nc.sync.dma_start(z[:], z_ext[b])
                nc.sync.dma_start(zdr[:], zdr_ext[b])

                out_sb = outp.tile([128, nt, D], bf16, tag="out")

                # two n-tiles per iteration: halves the per-op overhead on the
                # elementwise engines (psS spans 2 PSUM banks, one slice each)
                for p in range(nt // 2):
                    psS = psSp.tile([128, 2, K], f32, tag="psS")
                    for h in range(2):
                        n0 = (2 * p + h) * 128
                        # PSUM <- bias hi, then z1.c1 (f32r, exact), then the
                        # fp8 DoubleRow correction pass (z2.c1 + z1.c2 + bias lo)
                        nc.tensor.matmul(psS[:, h, :], ones[:], biasrow[:],
                                         start=True, stop=False)
                        for cd in range(2):
                            nc.tensor.matmul(psS[:, h, :], z[:, cd, n0:n0 + 128],
                                             ct[:, cd, :], start=False, stop=False)
                        for cd in range(2):
                            nc.tensor.matmul(
                                psS[:, h, :], zdr[:, cd, :, n0:n0 + 128],
                                cdr[:, cd, :, :], start=False, stop=(cd == 1),
                                perf_mode=mybir.MatmulPerfMode.DoubleRow)

                    # mx[n, h] = max_k S[n, h, k], straight from PSUM
                    mx = sco.tile([128, 2], f32, tag="mx")
                    nc.vector.reduce_max(mx[:], psS[:], axis=mybir.AxisListType.X)

                    # one-hot of the argmax, bf16
                    oh = sco.tile([128, 2, K], bf16, tag="oh")
                    for h in range(2):
                        nc.vector.tensor_scalar(
                            out=oh[:, h, :], in0=psS[:, h, :],
                            scalar1=mx[:, h:h + 1], scalar2=None,
                            op0=Alu.is_equal,
                        )

                    # transpose one-hot to [k, n] layout for the gather matmul
                    psT = psTp.tile([128, 2, K], bf16, tag="psT")
                    for h in range(2):
                        for j in range(4):
                            k0 = j * 128
                            nc.tensor.transpose(psT[:, h, k0:k0 + 128],
                                                oh[:, h, k0:k0 + 128], ident[:])
                    ohT = sco.tile([128, 2, K], bf16, tag="ohT")
                    nc.scalar.copy(ohT[:], psT[:])
                    psQ = psQp.tile([128, 2, D], f32, tag="psQ")
                    for h in range(2):
                        for j in range(4):
                            k0 = j * 128
                            nc.tensor.matmul(psQ[:, h, :],
                                             ohT[:, h, k0:k0 + 128],
                                             cbgb[:, j, :], start=(j == 0),
                                             stop=(j == 3))
                    nc.scalar.copy(out_sb[:, 2 * p:2 * p + 2, :], psQ[:])

                nc.sync.dma_start(out_ext[b], out_sb[:])

    nc.compile()
    return nc


def _get_nc():
    if "nc" not in _CACHE:
        _CACHE["nc"] = _build()
    return _CACHE["nc"]


def _r2n11(x):
    # Round f32 mantissas to 11 bits (nearest-even): the PE's float32r read
    # path keeps exactly 11 mantissa bits (hw-measured), so pre-rounded
    # operands make the f32r matmul exact.
    u = x.view(np.uint32).astype(np.uint64)
    rnd = np.uint64((1 << 11) - 1) + ((u >> np.uint64(12)) & np.uint64(1))
    return ((u + rnd) & ~np.uint64((1 << 12) - 1)).astype(np.uint32).view(np.float32)


def _prep_in_maps(z_e_x, c, embedding):
    import ml_dtypes

    bf = ml_dtypes.bfloat16
    f8 = ml_dtypes.float8_e4m3

    z = np.ascontiguousarray(np.asarray(z_e_x), dtype=np.float32)      # [B, D, H, W]
    cls = np.asarray(c).astype(np.int64)                               # [B]
    emb = np.ascontiguousarray(np.asarray(embedding), dtype=np.float32)

    # z in d-major [B, 2, 128, N] (d = cd*128 + p), split z = z1 + z2
    zf = np.ascontiguousarray(
        z.reshape(B, 2, 128, HH * WW))                                 # [B,2,128,N]
    z1 = _r2n11(zf)
    z2 = zf - z1
    zr = np.ascontiguousarray((z1 * np.float32(2.0 ** 9)).transpose(0, 2, 1, 3))
    # DR pairs: j=0 -> z2*2^10 (vs c1*2^8), j=1 -> z1*2^4 (vs c2*2^14)
    zdr = np.empty((B, 2, 2, 128, N), dtype=f8)
    zdr[:, :, 0] = (z2 * np.float32(2.0 ** 10)).astype(f8)
    zdr[:, :, 1] = (z1 * np.float32(2.0 ** 4)).astype(f8)

    cb = emb.reshape(NUM_CLASSES, K, D)[cls]                           # [B, 512, 256]
    cbT = np.ascontiguousarray(cb.transpose(0, 2, 1)).reshape(B, 2, 128, K)
    c1 = _r2n11(cbT)
    c2 = cbT - c1
    ct = np.ascontiguousarray((c1 * np.float32(2.0 ** 9)).transpose(0, 2, 1, 3))
    cdr = np.empty((B, 2, 2, 128, K), dtype=f8)
    cdr[:, :, 0] = (c1 * np.float32(2.0 ** 8)).astype(f8)
    cdr[:, :, 1] = (c2 * np.float32(2.0 ** 14)).astype(f8)

    bias = (-0.5 * 2.0 ** 18) * np.sum(cb.astype(np.float64) ** 2, axis=2)
    b11 = _r2n11(bias.astype(np.float32))                              # [B, 512]
    blo = (bias - b11.astype(np.float64)).astype(np.float32)
    # steal DR slot (cd=1, p=127, j=0): ones row on the z side, bias-lo on
    # the c side; drops the (tiny) z2.c1 contribution of d=255
    zdr[:, 1, 0, 127, :] = np.float32(1.0)
    cdr[:, 1, 0, 127, :] = blo.astype(f8)

    zdr = np.ascontiguousarray(zdr.transpose(0, 3, 1, 2, 4))           # [B,128,2,2,N]
    cdr = np.ascontiguousarray(cdr.transpose(0, 3, 1, 2, 4))           # [B,128,2,2,K]

    # gather operand: [B, 128, 4, 256] bf16 with k = j*128 + p
    cbgb = np.ascontiguousarray(
        cb.astype(bf).reshape(B, 4, 128, D).transpose(0, 2, 1, 3))

    biasrow = np.ascontiguousarray(b11[:, None, :])                    # [B, 1, 512]
    ones = np.ones((1, 128), dtype=np.float32)

    in_maps = []
    for i in range(NCORES):
        s = slice(i * BPC, (i + 1) * BPC)
        in_maps.append({
            "z": zr[s], "ct": ct[s], "zdr": zdr[s], "cdr": cdr[s],
            "cbgb": cbgb[s], "biasrow": biasrow[s], "ones": ones,
        })
    return in_maps


def kernel(z_e_x, c, embedding):
    from concourse.bass_utils import run_bass_kernel_spmd

    global LAST_EXEC_NS

    in_maps = _prep_in_maps(z_e_x, c, embedding)
    nc = _get_nc()
    res = run_bass_kernel_spmd(nc, in_maps, core_ids=list(range(NCORES)),
                               trace=TRACE)
    LAST_EXEC_NS = res.exec_time_ns

    outs = np.concatenate([res.results[i]["out"].astype(np.float32)
                           for i in range(NCORES)], axis=0)
    # [B, 128, NT, D] -> [B, N, D] with n = t*128 + p
    out = outs.transpose(0, 2, 1, 3).reshape(B, N, D)
    return np.ascontiguousarray(out.reshape(B, HH, WW, D))
